# revision 27
# baseline (speedup 1.0000x reference)
"""BoxE scorer kernel v2 for Trainium2 (8 NeuronCores, label-sharded).

"Widened-relu" formulation: with l1 = |x - cen|, hd = d/2,
m = relu(l1 - hd), s = [l1 > hd], the outside correction is
  corr_h = alpha*m^2 + beta*m + gamma*s .
Choosing w > 0 with  -alpha*w^2 + beta*w = gamma  (w = hd + c/(bb+a), always
positive here) and  rt = relu(l1 - hd + w):
  alpha*rt^2 + (beta - 2*alpha*w)*rt  ==  corr_h
exactly for points outside or deep inside; the band l1 in (hd-w, hd) picks
up a small spurious term bounded by gamma (measured end-to-end fro err
~4e-3 against the fp64 reference; budget 2e-2).

This removes both the s-plane and the m-plane of the classic 3-plane
decomposition: per label only
  rt' = relu(l1' - 1 + w')   (one dual-op DVE ts from l1', vec scalar bias)
  q   = rt'^2                (grouped ScalarE Square per 8-label group)
plus TWO PE matvecs per (label, chunk) and the base quadratic
sum_h a^2 (x-cen)^2 as 3 dense f32 matmuls.
"""

from contextlib import ExitStack

import numpy as np

import concourse.bass as bass
import concourse.tile as tile
from concourse import bacc, mybir
from concourse import bass_utils

F32 = mybir.dt.float32
F16 = mybir.dt.float16
BF16 = mybir.dt.bfloat16
U16 = mybir.dt.uint16
A = mybir.AluOpType
ACT = mybir.ActivationFunctionType

B = 1024      # batch
H = 128       # hidden
L = 2048      # num labels
N_CORES = 8
LPC = L // N_CORES   # labels per core
NBCH = B // 128      # batch chunks of 128
GRP = 8              # labels per grouped DVE instruction

N_DVE_L1 = 8  # l1-labels produced on DVE (rest: ScalarE ACT-Abs)
N_QTT = 0     # q-labels via DVE tensor_tensor (rest: ScalarE Square)
N_GPS_R = 0   # rt-labels on GPSIMD (slow; keep 0)


def build_nc(repeat: int = 1, n_dve_l1: int = N_DVE_L1, n_qtt: int = N_QTT,
             n_gps_r: int = N_GPS_R, ablate: frozenset = frozenset()):
    nc = bacc.Bacc("TRN2", target_bir_lowering=False, debug=False,
                   num_devices=N_CORES)
    xT_d = nc.dram_tensor("xT", (H, B), F32, kind="ExternalInput")
    mnT_d = nc.dram_tensor("mnT", (H, LPC), F32, kind="ExternalInput")
    rawT_d = nc.dram_tensor("rawT", (H, LPC), F32, kind="ExternalInput")
    out_d = nc.dram_tensor("out", (B, LPC), F32, kind="ExternalOutput")

    with tile.TileContext(nc) as tc:
        with ExitStack() as ctx:
            cpool = ctx.enter_context(tc.tile_pool(name="consts", bufs=1))
            pspool = ctx.enter_context(
                tc.tile_pool(name="psum", bufs=1, space=bass.MemorySpace.PSUM))

            # ---- load inputs ----
            ppool_cm = tc.tile_pool(name="pre", bufs=1)
            ppool = ppool_cm.__enter__()
            xT = cpool.tile([H, B], F32, tag="xT")
            nc.sync.dma_start(xT[:], xT_d.ap())
            mnT = ppool.tile([H, LPC], F32, tag="mnT")
            nc.sync.dma_start(mnT[:], mnT_d.ap())
            rawT = ppool.tile([H, LPC], F32, tag="rawT")
            nc.sync.dma_start(rawT[:], rawT_d.ap())

            def f32t(tag, pool=None):
                return (pool or cpool).tile([H, LPC], F32, tag=tag, name=tag)

            # ---- per-label coefficients (all [H, LPC] f32) ----
            e = f32t("e", pool=ppool)
            nc.scalar.activation(e[:], rawT[:], ACT.Exp)
            e1 = f32t("e1", pool=ppool)
            nc.vector.tensor_scalar(e1[:], e[:], 1.0, None, A.add)
            delta = f32t("delta", pool=ppool)     # softplus(raw)
            nc.scalar.activation(delta[:], e1[:], ACT.Ln)

            hd = f32t("hd", pool=ppool)          # d/2
            nc.vector.tensor_scalar(hd[:], delta[:], 0.5, None, A.mult)
            cen = f32t("cen", pool=ppool)        # mn + d/2
            nc.vector.tensor_tensor(cen[:], mnT[:], hd[:], A.add)
            invhd = f32t("invhd")                # 1/hd
            nc.vector.reciprocal(invhd[:], hd[:])
            cod = f32t("cod")                    # cen/hd
            nc.vector.tensor_tensor(cod[:], cen[:], invhd[:], A.mult)
            ncod = f32t("ncod")                  # -cen/hd (ACT-Abs bias)
            nc.vector.tensor_scalar(ncod[:], cod[:], -1.0, None, A.mult)

            dp1 = f32t("dp1", pool=ppool)        # bb = d+1
            nc.vector.tensor_scalar(dp1[:], delta[:], 1.0, None, A.add)
            dp1e = f32t("dp1e", pool=ppool)
            nc.vector.tensor_scalar(dp1e[:], dp1[:], 1e-10, None, A.add)
            a_ = f32t("a_", pool=ppool)          # a = 1/(bb+1e-10)
            nc.vector.reciprocal(a_[:], dp1e[:])
            de = f32t("de", pool=ppool)
            nc.vector.tensor_scalar(de[:], delta[:], 1e-10, None, A.add)
            rd = f32t("rd", pool=ppool)          # 1/(d+1e-10)
            nc.vector.reciprocal(rd[:], de[:])

            dmr = f32t("dmr", pool=ppool)        # d - 1/d
            nc.vector.tensor_tensor(dmr[:], delta[:], rd[:], A.subtract)
            nhd = f32t("nhd", pool=ppool)        # -d/2
            nc.vector.tensor_scalar(nhd[:], hd[:], -1.0, None, A.mult)
            c_ = f32t("c_", pool=ppool)          # c = -(d/2)(d - 1/d)
            nc.vector.tensor_tensor(c_[:], dmr[:], nhd[:], A.mult)

            Dl = f32t("Dl", pool=ppool)          # D = bb - a
            nc.vector.tensor_tensor(Dl[:], dp1[:], a_[:], A.subtract)
            abb = f32t("abb", pool=ppool)        # Q = bb + a
            nc.vector.tensor_tensor(abb[:], dp1[:], a_[:], A.add)
            al = f32t("al", pool=ppool)          # alpha = D*Q
            nc.vector.tensor_tensor(al[:], Dl[:], abb[:], A.mult)

            t2 = f32t("t2", pool=ppool)          # D + Q
            nc.vector.tensor_tensor(t2[:], Dl[:], abb[:], A.add)
            t3 = f32t("t3", pool=ppool)          # c*(D+Q)
            nc.vector.tensor_tensor(t3[:], t2[:], c_[:], A.mult)
            t4 = f32t("t4", pool=ppool)          # alpha*hd
            nc.vector.tensor_tensor(t4[:], al[:], hd[:], A.mult)
            t5 = f32t("t5", pool=ppool)          # 2*alpha*hd
            nc.vector.tensor_scalar(t5[:], t4[:], 2.0, None, A.mult)
            bp = f32t("bp", pool=ppool)          # beta
            nc.vector.tensor_tensor(bp[:], t5[:], t3[:], A.add)

            # w = hd + c/Q ;  w' = w/hd ; wm1 = w' - 1 (rt-op bias)
            rq = f32t("rq", pool=ppool)          # 1/Q
            nc.vector.reciprocal(rq[:], abb[:])
            cq_ = f32t("cq_", pool=ppool)        # c/Q
            nc.vector.tensor_tensor(cq_[:], c_[:], rq[:], A.mult)
            w_ = f32t("w_", pool=ppool)          # w = hd + c/Q
            nc.vector.tensor_tensor(w_[:], hd[:], cq_[:], A.add)
            wp = f32t("wp", pool=ppool)          # w' = w/hd
            nc.vector.tensor_tensor(wp[:], w_[:], invhd[:], A.mult)
            wm1 = f32t("wm1")                    # w' - 1
            nc.vector.tensor_scalar(wm1[:], wp[:], 1.0, None, A.subtract)

            # matvec coefficients: cq16 = alpha*hd^2 (bf16),
            # cr16 = (beta - 2*alpha*w)*hd (f16)
            ah = f32t("ah", pool=ppool)          # alpha*hd^2
            nc.vector.tensor_tensor(ah[:], t4[:], hd[:], A.mult)
            cq16 = cpool.tile([H, LPC], BF16, tag="cq16")
            nc.vector.tensor_copy(cq16[:], ah[:])
            taw = f32t("taw", pool=ppool)        # 2*alpha*w = t5*w'
            nc.vector.tensor_tensor(taw[:], t5[:], wp[:], A.mult)
            bw = f32t("bw", pool=ppool)          # beta - 2*alpha*w
            nc.vector.tensor_tensor(bw[:], bp[:], taw[:], A.subtract)
            bwh = f32t("bwh", pool=ppool)        # (beta-2*alpha*w)*hd
            nc.vector.tensor_tensor(bwh[:], bw[:], hd[:], A.mult)
            cr16 = cpool.tile([H, LPC], F16, tag="cr16")
            nc.vector.tensor_copy(cr16[:], bwh[:])

            # base-term planes (rhs of base matmuls), f32
            A2 = f32t("A2")                      # a^2
            nc.vector.tensor_tensor(A2[:], a_[:], a_[:], A.mult)
            acen = f32t("acen", pool=ppool)
            nc.vector.tensor_tensor(acen[:], a_[:], cen[:], A.mult)
            A2C2 = f32t("A2C2")                  # (a*cen)^2
            nc.vector.tensor_tensor(A2C2[:], acen[:], acen[:], A.mult)
            t6 = f32t("t6", pool=ppool)
            nc.vector.tensor_tensor(t6[:], A2[:], cen[:], A.mult)
            M2AC = f32t("M2AC")                  # -2*a^2*cen
            nc.vector.tensor_scalar(M2AC[:], t6[:], -2.0, None, A.mult)

            ppool_cm.__exit__(None, None, None)
            lpool = ctx.enter_context(tc.tile_pool(name="l1", bufs=2))
            rpool = ctx.enter_context(tc.tile_pool(name="rg", bufs=2))
            qpool = ctx.enter_context(tc.tile_pool(name="qg", bufs=2))
            # bufs=8: all 8 output chunks pipeline through sqrt/negate/
            # DMA concurrently (bufs=2 serialized the epilogue at
            # ~2.4us/chunk in the simulated timeline).
            opool = ctx.enter_context(tc.tile_pool(name="outs", bufs=8))
            x2T = cpool.tile([H, B], F32, tag="x2T")   # x^2
            nc.vector.tensor_tensor(x2T[:], xT[:], xT[:], A.mult)
            ones = cpool.tile([H, 128], F32, tag="ones")
            nc.gpsimd.memset(ones[:], 1.0)
            x16 = cpool.tile([H, B], F16, tag="x16")
            nc.vector.tensor_copy(x16[:], xT[:])

            tiles = dict(xT=xT, x2T=x2T, ones=ones, invhd=invhd, cod=cod,
                         ncod=ncod, wm1=wm1, x16=x16, A2=A2, M2AC=M2AC,
                         A2C2=A2C2, cq16=cq16, cr16=cr16)
            if repeat > 1:
                with tc.For_i(0, repeat, 1):
                    _run_body(nc, tc, lpool, rpool, qpool, pspool, opool,
                              tiles, out_d, n_dve_l1, n_qtt, n_gps_r, ablate)
            else:
                _run_body(nc, tc, lpool, rpool, qpool, pspool, opool,
                          tiles, out_d, n_dve_l1, n_qtt, n_gps_r, ablate)

    nc.compile()
    return nc


def _run_body(nc, tc, lpool, rpool, qpool, pspool, opool, tiles, out_d,
              n_dve_l1, n_qtt, n_gps_r, ablate=frozenset()):
    xT, x2T, ones = tiles["xT"], tiles["x2T"], tiles["ones"]
    invhd, cod, ncod = tiles["invhd"], tiles["cod"], tiles["ncod"]
    wm1, x16 = tiles["wm1"], tiles["x16"]
    A2, M2AC, A2C2 = tiles["A2"], tiles["M2AC"], tiles["A2C2"]
    cq16, cr16 = tiles["cq16"], tiles["cr16"]

    # ---- base matmuls into PSUM ----
    psts = []
    for cch in range(NBCH):
        pst = pspool.tile([128, LPC], F32, tag=f"ps{cch}")
        psts.append(pst)
        sl = bass.ts(cch, 128)
        nc.tensor.matmul(pst[:], x2T[:, sl], A2[:],
                         start=True, stop=False, skip_group_check=True)
        nc.tensor.matmul(pst[:], xT[:, sl], M2AC[:],
                         start=False, stop=False, skip_group_check=True)
        nc.tensor.matmul(pst[:], ones[:], A2C2[:],
                         start=False, stop=False, skip_group_check=True)

    # ---- per-label planes + PE reductions ----
    for g in range(LPC // GRP):
        l0 = g * GRP
        l1g = lpool.tile([H, GRP * B], F16, tag="l1g")
        for j in range(GRP):
            l = l0 + j
            lsl = slice(l, l + 1)
            gsl = slice(j * B, (j + 1) * B)
            if j < n_dve_l1:
                nc.vector.tensor_scalar(l1g[:, gsl], x16[:], invhd[:, lsl],
                                        cod[:, lsl], A.mult, A.subtract)
            else:
                nc.scalar.activation(l1g[:, gsl], xT[:], ACT.Abs,
                                     bias=ncod[:, lsl], scale=invhd[:, lsl])
        if n_dve_l1 > 0:
            nc.vector.tensor_scalar(
                l1g.bitcast(U16)[:, 0:n_dve_l1 * B],
                l1g.bitcast(U16)[:, 0:n_dve_l1 * B], 0x7FFF, None,
                A.bitwise_and)
        rg = rpool.tile([H, GRP * B], F16, tag="rg")
        for j in range(GRP):
            l = l0 + j
            lsl = slice(l, l + 1)
            gsl = slice(j * B, (j + 1) * B)
            eng = nc.gpsimd if j >= GRP - n_gps_r else nc.vector
            eng.tensor_scalar(rg[:, gsl], l1g[:, gsl], wm1[:, lsl], 0.0,
                              A.add, A.max)
        qg = qpool.tile([H, GRP * B], BF16, tag="qg")
        if n_qtt > 0:
            nc.vector.tensor_tensor(qg[:, :n_qtt * B], rg[:, :n_qtt * B],
                                    rg[:, :n_qtt * B], A.mult)
        if n_qtt < GRP:
            nc.scalar.activation(qg[:, n_qtt * B:], rg[:, n_qtt * B:],
                                 ACT.Square)

        if "pe" in ablate:
            continue
        for j in range(GRP):
            l = l0 + j
            lsl = slice(l, l + 1)
            last = l == LPC - 1
            for cch in range(NBCH):
                sl = slice(j * B + cch * 128, j * B + (cch + 1) * 128)
                pcol = psts[cch][:, lsl]
                nc.tensor.matmul(pcol, qg[:, sl], cq16[:, lsl],
                                 start=False, stop=False,
                                 skip_group_check=True)
                nc.tensor.matmul(pcol, rg[:, sl], cr16[:, lsl],
                                 start=False, stop=last,
                                 skip_group_check=True)
        if (g + 1) * GRP == LPC // 2:
            # Left half of every psum chunk is final: drain it now so
            # its sqrt/negate/DMA hide under the remaining groups.
            _epilogue(nc, opool, psts, out_d, 0, LPC // 2)

    # ---- finalize: out = -sqrt(psum); wave 0 was emitted mid-loop ----
    _epilogue(nc, opool, psts, out_d, LPC // 2, LPC)


def _epilogue(nc, opool, psts, out_d, c0, c1):
    n = c1 - c0
    for cch in range(NBCH):
        sq = opool.tile([128, n], F32, tag=f"sq{c0}", name=f"sq{c0}")
        nc.scalar.activation(sq[:], psts[cch][:, c0:c1], ACT.Sqrt)
        o = opool.tile([128, n], F32, tag=f"o{c0}", name=f"o{c0}")
        nc.vector.tensor_scalar(o[:], sq[:], -1.0, None, A.mult)
        nc.sync.dma_start(out_d.ap()[bass.ts(cch, 128), c0:c1], o[:])


_NC_CACHE = None


def _get_nc():
    global _NC_CACHE
    if _NC_CACHE is None:
        _NC_CACHE = build_nc()
    return _NC_CACHE


def kernel(y: np.ndarray, x: np.ndarray) -> np.ndarray:
    y = np.asarray(y, dtype=np.float32)
    x = np.asarray(x, dtype=np.float32)
    assert y.shape == (L, 2 * H) and x.shape == (B, H)

    nc = _get_nc()
    xT = np.ascontiguousarray(x.T)                       # (H, B)
    in_maps = []
    for c in range(N_CORES):
        ys = y[c * LPC:(c + 1) * LPC]
        in_maps.append({
            "xT": xT,
            "mnT": np.ascontiguousarray(ys[:, :H].T),    # (H, LPC)
            "rawT": np.ascontiguousarray(ys[:, H:].T),   # (H, LPC)
        })
    res = bass_utils.run_bass_kernel_spmd(nc, in_maps,
                                          core_ids=list(range(N_CORES)))
    out = np.concatenate([res.results[c]["out"] for c in range(N_CORES)],
                         axis=1)
    return np.ascontiguousarray(out.astype(np.float32))
